# revision 40
# baseline (speedup 1.0000x reference)
"""Trainium2 Bass kernel for the 3-group sparse attention module.

Shapes: x [4, 1024, 768], H=8 heads, head_dim 96 split into 3 groups of 32.
  qkv = x @ W_qkv -> q,k,v [B,H,N,96]; groups q3..q5/k3..k5/v3..v5 (32 each)
  x3 = attend(q4, [k3,k4], [v3,v4]); x4 = attend(q5, [k3,k5], [v3,v5])
  x5 = attend(q5, [k4,k5], [v4,v5]);  out = [x3|x4|x5] @ W_proj + b_proj
  scale = 96 ** -0.5

Sharding: 8 cores = 4 batches x 2 query-halves (no collectives).  Each core
computes k/v for the full sequence of its batch (all 8 heads) but queries /
attention / projection only for its 512 rows.  Host passes x transposed
(bf16) with the core's query rows first, so the SPMD graph is identical on
every core; key/value row order is consistently permuted which leaves
attention outputs unchanged.

Everything on-chip runs in "transposed activation space":
  qT/kT[d, n] from matmul(lhsT=W chunk, rhs=xT);  v[m, d] natural.
  S^T[m, n] = matmul(lhsT=kT[32, m-tile], rhs=qT[32, nq]) -- K=32 row-tiled.
  E = exp(scale * S^T) on ScalarE straight out of PSUM (scores are provably
  small: |s*scale| < ~1.2, so no max-subtraction pass is needed).
  y^T[d, n] = matmul(lhsT=[v|1][m-tile, 33], rhs=E) accumulated over m;
  row 32 of the PSUM then holds the softmax denominator Z for free.
  exp(q5 k5^T) @ [v5|1] is shared between x4 and x5 (computed once).
  proj: out[n, :] = matmul(lhsT=yT chunk, rhs=W_proj chunk) + bias.
"""

import numpy as np
import ml_dtypes

B, N, C, H = 4, 1024, 768, 8
HD = 96          # head dim
G = 32           # group dim
NQ = 512         # query rows per core
SCALE = float(HD) ** -0.5
P = 128
NCORES = 8

_CACHE = {}
# tuned configuration (measured best on TRN2):
AV_COL_TILING = False   # PE col-tiling for AV: slower (mode-switch drains)
E_FP8 = False           # fp8 probabilities fail the accuracy budget
AV_INTERLEAVE = True    # interleave the two AV accumulation streams
ST_INTERLEAVE = True    # round-robin score matmuls across PE row-bands
DEBUG_DUMP = False      # extra DRAM outputs for on-HW debugging


def _build_graph():
    import concourse.bass as bass
    import concourse.tile as tile
    from concourse import bacc, mybir

    f32 = mybir.dt.float32
    bf16 = mybir.dt.bfloat16
    i16 = mybir.dt.int16
    edt = mybir.dt.float8e4 if E_FP8 else mybir.dt.bfloat16
    # Schraudolph bf16-exp on DVE: bf16 bits of exp(t) ~= round(t*128/ln2
    # + 127*128 - c); c=7.65 zero-centers the relative error (rms ~1.7%).
    # Used to offload a few exp tiles per head from the Activation engine
    # (the per-head pipeline limiter) to the Vector engine.
    EXP_A = SCALE * 128.0 / float(np.log(2.0))
    EXP_B = 127.0 * 128.0 - 7.65

    nc = bacc.Bacc(num_devices=NCORES)

    xt_d = nc.declare_dram_parameter("xt", [C, NQ], bf16, isOutput=False)
    wq_d = nc.declare_dram_parameter("wq", [C, 768], bf16, isOutput=False)
    wk_d = nc.declare_dram_parameter("wk", [C, 768], bf16, isOutput=False)
    wv_d = nc.declare_dram_parameter("wv", [C, 768], bf16, isOutput=False)
    wp_d = nc.declare_dram_parameter("wp", [C, C], bf16, isOutput=False)
    bias_d = nc.declare_dram_parameter("bias", [P, C], f32, isOutput=False)
    out_d = nc.declare_dram_parameter("out", [NQ, C], f32, isOutput=True)
    if DEBUG_DUMP:
        udbg_d = nc.declare_dram_parameter("u_dbg", [C, NQ], bf16, isOutput=True)
        zdbg_d = nc.declare_dram_parameter("z_dbg", [288, NQ], f32, isOutput=True)

    # k/v exchange: each core computes k/v only for its OWN 512 rows and
    # AllGathers the pair's halves (cores 2b/2b+1 share batch b).  The
    # gathered m-order is [even core's rows | odd core's rows] for BOTH
    # cores -- attention is m-order invariant, so the SPMD graph stays
    # identical with no parity-dependent slot selection.
    RG = [[0, 1], [2, 3], [4, 5], [6, 7]]
    kstage_d = nc.dram_tensor("kxin", [C, NQ], bf16)
    kgath_d = nc.dram_tensor("kxout", [2 * C, NQ], bf16)
    vstage_d = nc.dram_tensor("vxin", [NQ, C], bf16)
    vgath_d = nc.dram_tensor("vxout", [2 * NQ, C], bf16)

    CH = C // P  # 6 chunks of 128 along the contraction/channel dims

    with tile.TileContext(nc) as tc:
        with (
            tc.tile_pool(name="wgt", bufs=1) as wgt,
            tc.tile_pool(name="acts", bufs=1) as acts,
            tc.tile_pool(name="epool", bufs=20) as epool,
            tc.tile_pool(name="small", bufs=2) as small,
            tc.tile_pool(name="outp", bufs=2) as outp,
            tc.tile_pool(name="psA", bufs=1, space="PSUM") as psA,
            tc.tile_pool(name="psB", bufs=3, space="PSUM") as psB,
        ):
            # ---- stage inputs in SBUF ----
            # Each dma_start costs ~565-667ns of SEQUENCER time before the
            # DMA engines see it; 30 serial issues on sync = 17us of dead
            # startup.  Spread issues across four engine sequencers (all
            # idle at t=0) and order by first use: xt q-half + wq unblock
            # gen_q(0); wk unblocks gen_k(0); wv unblocks gen_v.
            xt = [wgt.tile([P, NQ], bf16, name=f"xt{i}") for i in range(CH)]
            wq = [wgt.tile([P, 768], bf16, name=f"wq{i}") for i in range(CH)]
            wk = [wgt.tile([P, 768], bf16, name=f"wk{i}") for i in range(CH)]
            wv = [wgt.tile([P, 768], bf16, name=f"wv{i}") for i in range(CH)]
            wp = [wgt.tile([P, C], bf16, name=f"wp{i}") for i in range(CH)]
            bias = wgt.tile([P, C], f32, name="bias")
            for i in range(CH):
                nc.sync.dma_start(xt[i][:], xt_d[P * i:P * (i + 1), :])
            for i in range(CH):
                nc.sync.dma_start(wk[i][:], wk_d[P * i:P * (i + 1), :])
            for i in range(CH):
                nc.sync.dma_start(wq[i][:], wq_d[P * i:P * (i + 1), :])
            for i in range(CH):
                nc.gpsimd.dma_start(wv[i][:, 0:512], wv_d[P * i:P * (i + 1), 0:512])
            for i in range(CH):
                nc.gpsimd.dma_start(wv[i][:, 512:768], wv_d[P * i:P * (i + 1), 512:768])
            for i in range(CH):
                nc.gpsimd.dma_start(wp[i][:], wp_d[P * i:P * (i + 1), :])
            nc.gpsimd.dma_start(bias[:], bias_d[:])

            # ---- persistent activation tensors ----
            # qT: [768, 512]  per head h (96 rows at 96h): [q4; q5; q5]
            q_sb = [acts.tile([P, NQ], bf16, name=f"q{i}") for i in range(CH)]
            # kT: [768, 1024] per head: [k3; k5; k4]
            k_sb = [acts.tile([P, N], bf16, name=f"k{i}") for i in range(CH)]
            # qT copy #2 per head: band0 <- q5, band2 <- q4 (wave-2 blocks)
            q2_sb = [acts.tile([P, NQ], bf16, name=f"q2_{i}") for i in range(CH)]
            # v natural per m-tile: 24 groups of [1 x32 | v_g] (64 cols).
            # The 32 replicated ones-columns land the softmax denominator Z
            # on psum rows 0:32 of every AV accumulator, already broadcast
            # across 32 partitions -- normalize needs no gather/gpsimd
            # partition_broadcast.  M=33 -> 64 is free (stream time is N).
            # Z sits FIRST: reciprocal_approx_fast silently corrupts on HW
            # when its input AP has a non-zero partition offset.
            v_sb = [acts.tile([P, 24 * 64], bf16, name=f"v{i}") for i in range(8)]
            # unnormalized y^T (bf16) channels: 256*g + 32h + d
            u_sb = [acts.tile([P, NQ], bf16, name=f"u{i}") for i in range(CH)]

            psa_par = [0]

            def psa_tile(shape, name="ps"):
                tag = ("A", "B")[psa_par[0] % 2]
                psa_par[0] += 1
                return psA.tile(shape, f32, tag=tag, name=name), tag

            def band(h, j):
                """(tensor index, partition offset) of 32-row band j of head h."""
                p = 96 * h + 32 * j
                return p // P, p % P

            # ---- generation helpers (emitted piecemeal, interleaved with
            # attention so ScalarE starts exp-ing as early as possible) ----
            def gen_q(co):
                ps = psB.tile([P, NQ], f32, tag="av", name="qps")
                for ci in range(CH):
                    nc.tensor.matmul(
                        ps[:], lhsT=wq[ci][:, P * co:P * (co + 1)],
                        rhs=xt[ci][:, 0:NQ],
                        start=(ci == 0), stop=(ci == CH - 1))
                nc.vector.tensor_copy(q_sb[co][:], ps[:])

            kst = [wgt.tile([P, NQ], bf16, name=f"kst{i}") for i in range(CH)]
            vst = [wgt.tile([P, C], bf16, name=f"vst{i}") for i in range(4)]

            def gen_k(co):
                # own 512 rows only; staged to DRAM for the pair AllGather
                ps = psB.tile([P, NQ], f32, tag="av", name="kps")
                for ci in range(CH):
                    nc.tensor.matmul(
                        ps[:], lhsT=wk[ci][:, P * co:P * (co + 1)],
                        rhs=xt[ci][:, 0:NQ],
                        start=(ci == 0), stop=(ci == CH - 1))
                nc.vector.tensor_copy(kst[co][:], ps[:])
                nc.scalar.dma_start(kstage_d[P * co:P * (co + 1), :], kst[co][:])

            def gen_q2(h):
                # band2 <- q4 (q_sb band 0), band0 <- q5 (q_sb band 1)
                for dst_j, src_j in ((2, 0), (0, 1)):
                    dti, dpo = band(h, dst_j)
                    sti, spo = band(h, src_j)
                    nc.vector.tensor_copy(
                        q2_sb[dti][dpo:dpo + G, :], q_sb[sti][spo:spo + G, :])

            def gen_v(mt):
                # own 512 rows (m-tiles 0..3 of local xt), natural layout,
                # staged to DRAM for the pair AllGather
                for half, w in ((0, 512), (512, 256)):
                    ps = psB.tile([P, w], f32, tag="av", name="vps")
                    for ci in range(CH):
                        nc.tensor.matmul(
                            ps[:], lhsT=xt[ci][:, P * mt:P * (mt + 1)],
                            rhs=wv[ci][:, half:half + w],
                            start=(ci == 0), stop=(ci == CH - 1))
                    nc.vector.tensor_copy(vst[mt][:, half:half + w], ps[:])
                nc.scalar.dma_start(vstage_d[P * mt:P * (mt + 1), :], vst[mt][:])

            def kv_exchange():
                nc.gpsimd.collective_compute(
                    "AllGather", mybir.AluOpType.bypass, RG,
                    ins=[kstage_d[:]], outs=[kgath_d[:]])
                nc.gpsimd.collective_compute(
                    "AllGather", mybir.AluOpType.bypass, RG,
                    ins=[vstage_d[:]], outs=[vgath_d[:]])
                for co in range(CH):
                    for half in range(2):
                        nc.sync.dma_start(
                            k_sb[co][:, NQ * half:NQ * (half + 1)],
                            kgath_d[C * half + P * co:C * half + P * (co + 1), :])
                for mt in range(8):
                    slot, lt = mt // 4, mt % 4
                    src = vgath_d[NQ * slot + P * lt:NQ * slot + P * (lt + 1), :]
                    nc.sync.dma_start(
                        v_sb[mt][:].rearrange("p (g d) -> p g d", d=64)[:, :, 32:64],
                        src.rearrange("p (g d) -> p g d", d=32))

            # PE warmup: one matmul per arriving xt chunk keeps the HAM
            # clock-gate at 8/8 through the DMA window, so the gen phase
            # runs at 2.4GHz instead of cold 1.2.
            touch_ps, _ = psa_tile([P, NQ], name="touchps")
            for i in range(CH):
                nc.tensor.matmul(touch_ps[:], lhsT=xt[i][:, 0:P],
                                 rhs=xt[i][:, 0:NQ], start=True, stop=True)
                nc.tensor.matmul(touch_ps[:], lhsT=xt[i][:, P:2 * P],
                                 rhs=xt[i][:, 0:NQ], start=True, stop=True)

            # prologue: k/v gen + exchange feed every head, so they come
            # first; ones-memsets have no deps.  gen_q's are emitted at
            # low priority so the list scheduler uses them as PE filler
            # while the collective is in flight / attention stalls.
            for mt in range(8):
                nc.vector.memset(
                    v_sb[mt][:].rearrange("p (g d) -> p g d", d=64)[:, :, 0:32],
                    1.0)
            for co in range(CH):
                gen_k(co)
            for mt in range(4):
                gen_v(mt)
            kv_exchange()
            gen_q(0)
            gen_q2(0)

            with tc.high_priority(offset=-1000000):
                for co in range(1, CH):
                    gen_q(co)
                    for h2 in range(1, H):
                        if (96 * (h2 + 1) - 1) // P == co:
                            gen_q2(h2)

            # ---- attention per head ----
            # score blocks as (k tensor, band j, q band j2):
            #   wave1 (k_sb):  j0:(k3,q4)=Sa  j1:(k5,q5)=Sd  j2:(k4,q5)=Se
            #   wave2 (k2_sb): j0:(k4,q4)=Sb  j1:(k3,q5)=Sc
            # AV products accumulate into:
            #   y3 += Sa@[v3|1], Sb@[v4|1];  T = Sd@[v5|1]
            #   y4 += Sc@[v3|1] (+T);        y5 += Se@[v4|1] (+T)
            for h in range(H):
                # --- score matmuls, band-interleaved; psum tiles pack 3
                # (tag A) or 2 (tag B) results -> one exp each, amortising
                # the ~352-cycle ACT per-instruction overhead (128 exps
                # instead of 160).
                e_map = {}
                seq = [(name, kj, qj, qsrc, mt)
                       for mt in range(8)
                       for name, kj, qj, qsrc in
                       (("a", 0, 0, q_sb), ("d", 1, 1, q_sb),
                        ("e", 2, 2, q_sb))]
                seq += [(name, kj, qj, qsrc, mt)
                        for mt in range(8)
                        for name, kj, qj, qsrc in
                        (("b", 2, 2, q2_sb), ("c", 0, 0, q2_sb))]
                ps, fill, pack = None, 0, 0
                w2i = 0  # index of completed wave-2 (2-wide) packs
                for name, kj, qj, qsrc, mt in seq:
                    if ps is None:
                        ps, tag = psa_tile([P, (3 if psa_par[0] % 2 == 0
                                                else 2) * NQ], name="sps")
                        pack = 3 if tag == "A" else 2
                        fill = 0
                    kti, kpo = band(h, kj)
                    qti, qpo = band(h, qj)
                    nc.tensor.matmul(
                        ps[:, NQ * fill:NQ * (fill + 1)],
                        lhsT=k_sb[kti][kpo:kpo + G, P * mt:P * (mt + 1)],
                        rhs=qsrc[qti][qpo:qpo + G, :],
                        start=True, stop=True,
                        tile_position=(kpo, 0))
                    e_map[(name, mt)] = (None, fill)
                    fill += 1
                    if fill == pack:
                        et = epool.tile([P, 3 * NQ], edt, tag="e")
                        # offload 3 of the 8 wave-2 packs per head to the
                        # Vector engine (Schraudolph bf16 exp, ~1.7% rms)
                        # to rebalance: ACT is the per-head limiter.
                        use_dve = pack == 2 and w2i in (2, 4, 6)
                        if pack == 2:
                            w2i += 1
                        if use_dve:
                            nc.vector.tensor_scalar(
                                et[:, 0:pack * NQ].bitcast(i16),
                                ps[:, 0:pack * NQ], EXP_A, EXP_B,
                                mybir.AluOpType.mult, mybir.AluOpType.add)
                        else:
                            nc.scalar.activation(
                                et[:, 0:pack * NQ], ps[:, 0:pack * NQ],
                                mybir.ActivationFunctionType.Exp, scale=SCALE)
                        for k2, v2 in e_map.items():
                            if v2[0] is None:
                                e_map[k2] = (et, v2[1])
                        ps = None

                def e_rhs(name, mt):
                    et, sl = e_map[(name, mt)]
                    return et[:, NQ * sl:NQ * (sl + 1)]

                # --- AV: y3 = a+b, y4 = c+T, y5 = e+T where T = Sd@[1|v5]
                # is the block shared by x4/x5, accumulated ONCE (8 fewer
                # matmuls/head than double-accumulating d).  ps_y4's bank
                # holds d first (snapshot to t_sb), then restarts with c.
                ps_y3 = psB.tile([P, NQ], f32, tag="av")
                ps_y4 = psB.tile([P, NQ], f32, tag="av")
                ps_y5 = psB.tile([P, NQ], f32, tag="av")

                def av_mm(acc, name, vg, mt, i, n):
                    gg = 3 * h + vg
                    nc.tensor.matmul(
                        acc[0:64, :],
                        lhsT=v_sb[mt][:, 64 * gg:64 * gg + 64],
                        rhs=e_rhs(name, mt),
                        start=(i == 0), stop=(i == n - 1))

                for mt in range(8):
                    av_mm(ps_y3, "a", 0, mt, mt, 16)
                    av_mm(ps_y4, "d", 2, mt, mt, 8)
                    av_mm(ps_y5, "e", 1, mt, mt, 8)

                # --- normalize: gather y rows / Z rows of the 3 groups
                # into [96, 512] tiles at partition offset 0 (the y4/y5
                # gathers double as the +T adds), then ONE approx-recip +
                # ONE mul per head.  Constraints honoured: tensor_tensor
                # inputs must share a start partition (and only one may be
                # PSUM), and the custom-DVE recip only works with offset-0
                # APs on HW (sim diverges).  u channels are head-contiguous
                # (96h + 32g + d); the host permutes W_proj rows to match.
                yg = small.tile([96, NQ], f32, tag="yg")
                zg = small.tile([96, NQ], f32, tag="zg")
                t_sb = small.tile([64, NQ], f32, tag="tsb")
                nc.vector.tensor_copy(t_sb[:], ps_y4[0:64, :])
                nc.vector.tensor_add(yg[64:96, :], ps_y5[32:64, :],
                                     t_sb[32:64, :])
                nc.vector.tensor_add(zg[64:96, :], ps_y5[0:32, :],
                                     t_sb[0:32, :])
                # b's first so the t_sb copy (WAR on ps_y4) is off the PE
                # critical path when the c accumulation restarts the bank.
                for mt in range(8):
                    av_mm(ps_y3, "b", 1, mt, 8 + mt, 16)
                for mt in range(8):
                    av_mm(ps_y4, "c", 0, mt, mt, 8)
                nc.vector.tensor_copy(yg[0:32, :], ps_y3[32:64, :])
                nc.vector.tensor_copy(zg[0:32, :], ps_y3[0:32, :])
                nc.vector.tensor_add(yg[32:64, :], ps_y4[32:64, :],
                                     t_sb[32:64, :])
                nc.vector.tensor_add(zg[32:64, :], ps_y4[0:32, :],
                                     t_sb[0:32, :])
                rz = small.tile([96, NQ], f32, tag="rzg")
                nc.vector.reciprocal_approx_fast(rz[:], zg[:])
                po = (96 * h) % P
                if po == 0:
                    nc.vector.tensor_mul(
                        u_sb[(96 * h) // P][0:96, :], yg[:], rz[:])
                else:
                    # partition rule: an engine AP starting at partition
                    # 32/64/96 spans at most 32 partitions -> 32-row muls.
                    for g in range(3):
                        ch = 96 * h + 32 * g
                        nc.vector.tensor_mul(
                            u_sb[ch // P][ch % P:ch % P + G, :],
                            yg[32 * g:32 * g + G, :],
                            rz[32 * g:32 * g + G, :])
                if DEBUG_DUMP and h == 7:
                    nc.sync.dma_start(zdbg_d[0:96, :], rz[:])
                    nc.sync.dma_start(zdbg_d[96:192, :], yg[:])
                    nc.sync.dma_start(zdbg_d[192:288, :], zg[:])

            if DEBUG_DUMP:
                for ci in range(CH):
                    nc.sync.dma_start(udbg_d[P * ci:P * (ci + 1), :],
                                      u_sb[ci][:])

            # ---- projection + bias ----
            for nt in range(4):
                ps, _ = psa_tile([P, C], name="pps")
                for half, w in ((0, 512), (512, 256)):
                    for ci in range(CH):
                        nc.tensor.matmul(
                            ps[:, half:half + w],
                            lhsT=u_sb[ci][:, P * nt:P * (nt + 1)],
                            rhs=wp[ci][:, half:half + w],
                            start=(ci == 0), stop=(ci == CH - 1))
                o_sb = outp.tile([P, C], f32, tag="osb")
                nc.vector.tensor_add(o_sb[:], ps[:], bias[:])
                nc.sync.dma_start(out_d[P * nt:P * (nt + 1), :], o_sb[:])

    nc.finalize()
    return nc


def _prep_inputs(x, W_qkv, W_proj, b_proj):
    bf16 = ml_dtypes.bfloat16
    # wq: per head [q4, q5, q5] (96 cols); wk: per head [k3, k5, k4]
    qcols, kcols = [], []
    for h in range(H):
        qb, kb = HD * h, C + HD * h
        qcols += list(range(qb + 32, qb + 64)) + 2 * list(range(qb + 64, qb + 96))
        kcols += (list(range(kb, kb + 32)) + list(range(kb + 64, kb + 96))
                  + list(range(kb + 32, kb + 64)))
    wq = np.ascontiguousarray(W_qkv[:, qcols]).astype(bf16)
    wk = np.ascontiguousarray(W_qkv[:, kcols]).astype(bf16)
    wv = np.ascontiguousarray(W_qkv[:, 2 * C:3 * C]).astype(bf16)
    # u channels on-chip are head-contiguous (96h + 32g + d); permute the
    # proj weight rows to match: wp[96h + 32g + d] = W_proj[256g + 32h + d].
    uperm = np.empty(C, np.int64)
    for hh in range(H):
        for g in range(3):
            uperm[96 * hh + 32 * g:96 * hh + 32 * g + 32] = np.arange(
                256 * g + 32 * hh, 256 * g + 32 * hh + 32)
    wp = np.ascontiguousarray(np.asarray(W_proj)[uperm, :]).astype(bf16)
    bias = np.broadcast_to(np.asarray(b_proj, np.float32), (P, C)).copy()

    in_maps = []
    for core in range(NCORES):
        b, half = core // 2, core % 2
        xb = np.asarray(x[b], np.float32)
        # own 512 query rows only; the pair AllGather supplies k/v for the
        # full sequence in [even-core rows | odd-core rows] order.
        xp = xb[NQ * half:NQ * (half + 1)]
        xt = np.ascontiguousarray(xp.T).astype(bf16)
        in_maps.append({"xt": xt, "wq": wq, "wk": wk, "wv": wv, "wp": wp,
                        "bias": bias})
    return in_maps


def kernel(x, W_qkv, W_proj, b_proj, t_h=None, t_w=None, s_h=None, s_w=None,
           **_unused):
    from concourse.bass_utils import run_bass_kernel_spmd

    if "nc" not in _CACHE:
        _CACHE["nc"] = _build_graph()
    nc = _CACHE["nc"]

    in_maps = _prep_inputs(np.asarray(x), np.asarray(W_qkv),
                           np.asarray(W_proj), np.asarray(b_proj))
    res = run_bass_kernel_spmd(nc, in_maps, core_ids=list(range(NCORES)))
    _CACHE["last_results"] = res

    out = np.empty((B, N, C), np.float32)
    for core in range(NCORES):
        b, half = core // 2, core % 2
        out[b, NQ * half:NQ * (half + 1), :] = res.results[core]["out"]
    return out



# revision 42
# speedup vs baseline: 1.1512x; 1.1512x over previous
"""Trainium2 Bass kernel for the 3-group sparse attention module.

Shapes: x [4, 1024, 768], H=8 heads, head_dim 96 split into 3 groups of 32.
  qkv = x @ W_qkv -> q,k,v [B,H,N,96]; groups q3..q5/k3..k5/v3..v5 (32 each)
  x3 = attend(q4, [k3,k4], [v3,v4]); x4 = attend(q5, [k3,k5], [v3,v5])
  x5 = attend(q5, [k4,k5], [v4,v5]);  out = [x3|x4|x5] @ W_proj + b_proj
  scale = 96 ** -0.5

Sharding: 8 cores = 4 batches x 2 query-halves (no collectives).  Each core
computes k/v for the full sequence of its batch (all 8 heads) but queries /
attention / projection only for its 512 rows.  Host passes x transposed
(bf16) with the core's query rows first, so the SPMD graph is identical on
every core; key/value row order is consistently permuted which leaves
attention outputs unchanged.

Everything on-chip runs in "transposed activation space":
  qT/kT[d, n] from matmul(lhsT=W chunk, rhs=xT);  v[m, d] natural.
  S^T[m, n] = matmul(lhsT=kT[32, m-tile], rhs=qT[32, nq]) -- K=32 row-tiled.
  E = exp(scale * S^T) straight out of PSUM (scores are provably small:
  |s*scale| < ~1.2, so no max-subtraction pass is needed); most exp tiles
  run on ScalarE, 3 of 8 wave-2 packs per head on VectorE via a
  Schraudolph bf16 bit-trick (one fused tensor_scalar) to debottleneck ACT.
  y^T[d, n] = matmul(lhsT=[1x32|v][m-tile, 64], rhs=E) accumulated over m;
  psum rows 0:32 then hold the softmax denominator Z, replicated across 32
  partitions (no gather / partition-broadcast needed for normalize).
  T = exp(q5 k5^T) @ [1|v5] is shared between x4 and x5: accumulated once
  (ps_y4 holds d, snapshot to SBUF, bank restarts with c), added into
  y4/y5 by the same DVE ops that gather y/Z into offset-0 [96, 512] tiles.
  normalize: one reciprocal_approx_fast + one (or three) muls per head.
  proj: out[n, :] = matmul(lhsT=u chunk, rhs=W_proj chunk) + bias, with
  W_proj rows host-permuted to the head-contiguous u channel order.

HW quirks honoured (all found empirically on TRN2; CoreSim diverges):
  - custom-DVE ops corrupt unless ALL APs start at partition 0
  - tensor_tensor inputs must share a start partition; at most one PSUM
  - GPSIMD cannot access PSUM; DMA only issues from SP/Activation/GPSIMD
"""

import numpy as np
import ml_dtypes

B, N, C, H = 4, 1024, 768, 8
HD = 96          # head dim
G = 32           # group dim
NQ = 512         # query rows per core
SCALE = float(HD) ** -0.5
P = 128
NCORES = 8

_CACHE = {}
# tuned configuration (measured best on TRN2):
AV_COL_TILING = False   # PE col-tiling for AV: slower (mode-switch drains)
E_FP8 = False           # fp8 probabilities fail the accuracy budget
AV_INTERLEAVE = True    # interleave the two AV accumulation streams
ST_INTERLEAVE = True    # round-robin score matmuls across PE row-bands
DEBUG_DUMP = False      # extra DRAM outputs for on-HW debugging


def _build_graph():
    import concourse.bass as bass
    import concourse.tile as tile
    from concourse import bacc, mybir

    f32 = mybir.dt.float32
    bf16 = mybir.dt.bfloat16
    i16 = mybir.dt.int16
    edt = mybir.dt.float8e4 if E_FP8 else mybir.dt.bfloat16
    # Schraudolph bf16-exp on DVE: bf16 bits of exp(t) ~= round(t*128/ln2
    # + 127*128 - c); c=7.65 zero-centers the relative error (rms ~1.7%).
    # Used to offload a few exp tiles per head from the Activation engine
    # (the per-head pipeline limiter) to the Vector engine.
    EXP_A = SCALE * 128.0 / float(np.log(2.0))
    EXP_B = 127.0 * 128.0 - 7.65

    nc = bacc.Bacc()

    xt_d = nc.declare_dram_parameter("xt", [C, N], bf16, isOutput=False)
    wq_d = nc.declare_dram_parameter("wq", [C, 768], bf16, isOutput=False)
    wk_d = nc.declare_dram_parameter("wk", [C, 768], bf16, isOutput=False)
    wv_d = nc.declare_dram_parameter("wv", [C, 768], bf16, isOutput=False)
    wp_d = nc.declare_dram_parameter("wp", [C, C], bf16, isOutput=False)
    bias_d = nc.declare_dram_parameter("bias", [P, C], f32, isOutput=False)
    out_d = nc.declare_dram_parameter("out", [NQ, C], f32, isOutput=True)
    if DEBUG_DUMP:
        udbg_d = nc.declare_dram_parameter("u_dbg", [C, NQ], bf16, isOutput=True)
        zdbg_d = nc.declare_dram_parameter("z_dbg", [288, NQ], f32, isOutput=True)

    CH = C // P  # 6 chunks of 128 along the contraction/channel dims

    with tile.TileContext(nc) as tc:
        with (
            tc.tile_pool(name="wgt", bufs=1) as wgt,
            tc.tile_pool(name="acts", bufs=1) as acts,
            tc.tile_pool(name="epool", bufs=20) as epool,
            tc.tile_pool(name="small", bufs=2) as small,
            tc.tile_pool(name="outp", bufs=2) as outp,
            tc.tile_pool(name="psA", bufs=1, space="PSUM") as psA,
            tc.tile_pool(name="psB", bufs=3, space="PSUM") as psB,
        ):
            # ---- stage inputs in SBUF ----
            # Each dma_start costs ~565-667ns of SEQUENCER time before the
            # DMA engines see it; 30 serial issues on sync = 17us of dead
            # startup.  Spread issues across four engine sequencers (all
            # idle at t=0) and order by first use: xt q-half + wq unblock
            # gen_q(0); wk unblocks gen_k(0); wv unblocks gen_v.
            xt = [wgt.tile([P, N], bf16, name=f"xt{i}") for i in range(CH)]
            wq = [wgt.tile([P, 768], bf16, name=f"wq{i}") for i in range(CH)]
            wk = [wgt.tile([P, 768], bf16, name=f"wk{i}") for i in range(CH)]
            wv = [wgt.tile([P, 768], bf16, name=f"wv{i}") for i in range(CH)]
            wp = [wgt.tile([P, C], bf16, name=f"wp{i}") for i in range(CH)]
            bias = wgt.tile([P, C], f32, name="bias")
            for i in range(CH):
                nc.sync.dma_start(xt[i][:, 0:NQ], xt_d[P * i:P * (i + 1), 0:NQ])
            for i in range(CH):
                nc.sync.dma_start(wq[i][:], wq_d[P * i:P * (i + 1), :])
            for i in range(CH):
                nc.sync.dma_start(xt[i][:, NQ:N], xt_d[P * i:P * (i + 1), NQ:N])
                nc.sync.dma_start(wk[i][:], wk_d[P * i:P * (i + 1), :])
            for i in range(CH):
                nc.sync.dma_start(wv[i][:, 0:512], wv_d[P * i:P * (i + 1), 0:512])
            for i in range(CH):
                nc.sync.dma_start(wv[i][:, 512:768], wv_d[P * i:P * (i + 1), 512:768])
            for i in range(CH):
                nc.sync.dma_start(wp[i][:], wp_d[P * i:P * (i + 1), :])
            nc.sync.dma_start(bias[:], bias_d[:])

            # ---- persistent activation tensors ----
            # qT: [768, 512]  per head h (96 rows at 96h): [q4; q5; q5]
            q_sb = [acts.tile([P, NQ], bf16, name=f"q{i}") for i in range(CH)]
            # kT: [768, 1024] per head: [k3; k5; k4]
            k_sb = [acts.tile([P, N], bf16, name=f"k{i}") for i in range(CH)]
            # qT copy #2 per head: band0 <- q5, band2 <- q4 (wave-2 blocks)
            q2_sb = [acts.tile([P, NQ], bf16, name=f"q2_{i}") for i in range(CH)]
            # v natural per m-tile: 24 groups of [1 x32 | v_g] (64 cols).
            # The 32 replicated ones-columns land the softmax denominator Z
            # on psum rows 0:32 of every AV accumulator, already broadcast
            # across 32 partitions -- normalize needs no gather/gpsimd
            # partition_broadcast.  M=33 -> 64 is free (stream time is N).
            # Z sits FIRST: reciprocal_approx_fast silently corrupts on HW
            # when its input AP has a non-zero partition offset.
            v_sb = [acts.tile([P, 24 * 64], bf16, name=f"v{i}") for i in range(8)]
            # unnormalized y^T (bf16) channels: 256*g + 32h + d
            u_sb = [acts.tile([P, NQ], bf16, name=f"u{i}") for i in range(CH)]

            psa_par = [0]

            def psa_tile(shape, name="ps"):
                tag = ("A", "B")[psa_par[0] % 2]
                psa_par[0] += 1
                return psA.tile(shape, f32, tag=tag, name=name), tag

            def band(h, j):
                """(tensor index, partition offset) of 32-row band j of head h."""
                p = 96 * h + 32 * j
                return p // P, p % P

            # ---- generation helpers (emitted piecemeal, interleaved with
            # attention so ScalarE starts exp-ing as early as possible) ----
            def gen_q(co):
                ps = psB.tile([P, NQ], f32, tag="av", name="qps")
                for ci in range(CH):
                    nc.tensor.matmul(
                        ps[:], lhsT=wq[ci][:, P * co:P * (co + 1)],
                        rhs=xt[ci][:, 0:NQ],
                        start=(ci == 0), stop=(ci == CH - 1))
                nc.vector.tensor_copy(q_sb[co][:], ps[:])

            def gen_k(co):
                for nh in range(2):
                    ps = psB.tile([P, NQ], f32, tag="av", name="kps")
                    for ci in range(CH):
                        nc.tensor.matmul(
                            ps[:], lhsT=wk[ci][:, P * co:P * (co + 1)],
                            rhs=xt[ci][:, NQ * nh:NQ * (nh + 1)],
                            start=(ci == 0), stop=(ci == CH - 1))
                    nc.vector.tensor_copy(k_sb[co][:, NQ * nh:NQ * (nh + 1)], ps[:])

            def gen_q2(h):
                # band2 <- q4 (q_sb band 0), band0 <- q5 (q_sb band 1)
                for dst_j, src_j in ((2, 0), (0, 1)):
                    dti, dpo = band(h, dst_j)
                    sti, spo = band(h, src_j)
                    nc.vector.tensor_copy(
                        q2_sb[dti][dpo:dpo + G, :], q_sb[sti][spo:spo + G, :])

            def gen_v(mt):
                vdst = v_sb[mt][:].rearrange("p (g d) -> p g d", d=64)
                for half, w, g0 in ((0, 512, 0), (512, 256, 16)):
                    ps = psB.tile([P, w], f32, tag="av", name="vps")
                    for ci in range(CH):
                        nc.tensor.matmul(
                            ps[:], lhsT=xt[ci][:, P * mt:P * (mt + 1)],
                            rhs=wv[ci][:, half:half + w],
                            start=(ci == 0), stop=(ci == CH - 1))
                    nc.vector.tensor_copy(
                        vdst[:, g0:g0 + w // G, 32:64],
                        ps[:].rearrange("p (g d) -> p g d", d=32))
                nc.vector.memset(vdst[:, :, 0:32], 1.0)

            # PE warmup: one matmul per arriving xt chunk keeps the HAM
            # clock-gate at 8/8 through the DMA window, so the gen phase
            # runs at 2.4GHz instead of cold 1.2.
            touch_ps, _ = psa_tile([P, NQ], name="touchps")
            for i in range(CH):
                nc.tensor.matmul(touch_ps[:], lhsT=xt[i][:, 0:P],
                                 rhs=xt[i][:, 0:NQ], start=True, stop=True)
                nc.tensor.matmul(touch_ps[:], lhsT=xt[i][:, P:2 * P],
                                 rhs=xt[i][:, 0:NQ], start=True, stop=True)

            # prologue: just enough for head 0; everything else is emitted
            # AFTER the attention chain (= lower scheduler priority) so the
            # list scheduler uses it as PE filler whenever attention stalls.
            gen_q(0)
            gen_k(0)
            gen_q2(0)

            with tc.high_priority(offset=-1000000):
                gen_k(1); gen_q(1); gen_q2(1)
                for mt in range(8):
                    gen_v(mt)
                for co in range(2, CH):
                    gen_q(co); gen_k(co)
                    for h2 in range(2, H):
                        if (96 * (h2 + 1) - 1) // P == co:
                            gen_q2(h2)

            # ---- attention per head ----
            # score blocks as (k tensor, band j, q band j2):
            #   wave1 (k_sb):  j0:(k3,q4)=Sa  j1:(k5,q5)=Sd  j2:(k4,q5)=Se
            #   wave2 (k2_sb): j0:(k4,q4)=Sb  j1:(k3,q5)=Sc
            # AV products accumulate into:
            #   y3 += Sa@[v3|1], Sb@[v4|1];  T = Sd@[v5|1]
            #   y4 += Sc@[v3|1] (+T);        y5 += Se@[v4|1] (+T)
            for h in range(H):
                # --- score matmuls, band-interleaved; psum tiles pack 3
                # (tag A) or 2 (tag B) results -> one exp each, amortising
                # the ~352-cycle ACT per-instruction overhead (128 exps
                # instead of 160).
                e_map = {}
                seq = [(name, kj, qj, qsrc, mt)
                       for mt in range(8)
                       for name, kj, qj, qsrc in
                       (("a", 0, 0, q_sb), ("d", 1, 1, q_sb),
                        ("e", 2, 2, q_sb))]
                seq += [(name, kj, qj, qsrc, mt)
                        for mt in range(8)
                        for name, kj, qj, qsrc in
                        (("b", 2, 2, q2_sb), ("c", 0, 0, q2_sb))]
                ps, fill, pack = None, 0, 0
                w2i = 0  # index of completed wave-2 (2-wide) packs
                for name, kj, qj, qsrc, mt in seq:
                    if ps is None:
                        ps, tag = psa_tile([P, (3 if psa_par[0] % 2 == 0
                                                else 2) * NQ], name="sps")
                        pack = 3 if tag == "A" else 2
                        fill = 0
                    kti, kpo = band(h, kj)
                    qti, qpo = band(h, qj)
                    nc.tensor.matmul(
                        ps[:, NQ * fill:NQ * (fill + 1)],
                        lhsT=k_sb[kti][kpo:kpo + G, P * mt:P * (mt + 1)],
                        rhs=qsrc[qti][qpo:qpo + G, :],
                        start=True, stop=True,
                        tile_position=(kpo, 0))
                    e_map[(name, mt)] = (None, fill)
                    fill += 1
                    if fill == pack:
                        et = epool.tile([P, 3 * NQ], edt, tag="e")
                        # offload 3 of the 8 wave-2 packs per head to the
                        # Vector engine (Schraudolph bf16 exp, ~1.7% rms)
                        # to rebalance: ACT is the per-head limiter.
                        use_dve = pack == 2 and w2i in (2, 4, 6)
                        if pack == 2:
                            w2i += 1
                        if use_dve:
                            nc.vector.tensor_scalar(
                                et[:, 0:pack * NQ].bitcast(i16),
                                ps[:, 0:pack * NQ], EXP_A, EXP_B,
                                mybir.AluOpType.mult, mybir.AluOpType.add)
                        else:
                            nc.scalar.activation(
                                et[:, 0:pack * NQ], ps[:, 0:pack * NQ],
                                mybir.ActivationFunctionType.Exp, scale=SCALE)
                        for k2, v2 in e_map.items():
                            if v2[0] is None:
                                e_map[k2] = (et, v2[1])
                        ps = None

                def e_rhs(name, mt):
                    et, sl = e_map[(name, mt)]
                    return et[:, NQ * sl:NQ * (sl + 1)]

                # --- AV: y3 = a+b, y4 = c+T, y5 = e+T where T = Sd@[1|v5]
                # is the block shared by x4/x5, accumulated ONCE (8 fewer
                # matmuls/head than double-accumulating d).  ps_y4's bank
                # holds d first (snapshot to t_sb), then restarts with c.
                ps_y3 = psB.tile([P, NQ], f32, tag="av")
                ps_y4 = psB.tile([P, NQ], f32, tag="av")
                ps_y5 = psB.tile([P, NQ], f32, tag="av")

                def av_mm(acc, name, vg, mt, i, n):
                    gg = 3 * h + vg
                    nc.tensor.matmul(
                        acc[0:64, :],
                        lhsT=v_sb[mt][:, 64 * gg:64 * gg + 64],
                        rhs=e_rhs(name, mt),
                        start=(i == 0), stop=(i == n - 1))

                for mt in range(8):
                    av_mm(ps_y3, "a", 0, mt, mt, 16)
                    av_mm(ps_y4, "d", 2, mt, mt, 8)
                    av_mm(ps_y5, "e", 1, mt, mt, 8)

                # --- normalize: gather y rows / Z rows of the 3 groups
                # into [96, 512] tiles at partition offset 0 (the y4/y5
                # gathers double as the +T adds), then ONE approx-recip +
                # ONE mul per head.  Constraints honoured: tensor_tensor
                # inputs must share a start partition (and only one may be
                # PSUM), and the custom-DVE recip only works with offset-0
                # APs on HW (sim diverges).  u channels are head-contiguous
                # (96h + 32g + d); the host permutes W_proj rows to match.
                yg = small.tile([96, NQ], f32, tag="yg")
                zg = small.tile([96, NQ], f32, tag="zg")
                t_sb = small.tile([64, NQ], f32, tag="tsb")
                nc.vector.tensor_copy(t_sb[:], ps_y4[0:64, :])
                nc.vector.tensor_add(yg[64:96, :], ps_y5[32:64, :],
                                     t_sb[32:64, :])
                nc.vector.tensor_add(zg[64:96, :], ps_y5[0:32, :],
                                     t_sb[0:32, :])
                # b's first so the t_sb copy (WAR on ps_y4) is off the PE
                # critical path when the c accumulation restarts the bank.
                for mt in range(8):
                    av_mm(ps_y3, "b", 1, mt, 8 + mt, 16)
                for mt in range(8):
                    av_mm(ps_y4, "c", 0, mt, mt, 8)
                nc.vector.tensor_copy(yg[0:32, :], ps_y3[32:64, :])
                nc.vector.tensor_copy(zg[0:32, :], ps_y3[0:32, :])
                nc.vector.tensor_add(yg[32:64, :], ps_y4[32:64, :],
                                     t_sb[32:64, :])
                nc.vector.tensor_add(zg[32:64, :], ps_y4[0:32, :],
                                     t_sb[0:32, :])
                rz = small.tile([96, NQ], f32, tag="rzg")
                nc.vector.reciprocal_approx_fast(rz[:], zg[:])
                po = (96 * h) % P
                if po == 0:
                    nc.vector.tensor_mul(
                        u_sb[(96 * h) // P][0:96, :], yg[:], rz[:])
                else:
                    # partition rule: an engine AP starting at partition
                    # 32/64/96 spans at most 32 partitions -> 32-row muls.
                    for g in range(3):
                        ch = 96 * h + 32 * g
                        nc.vector.tensor_mul(
                            u_sb[ch // P][ch % P:ch % P + G, :],
                            yg[32 * g:32 * g + G, :],
                            rz[32 * g:32 * g + G, :])
                if DEBUG_DUMP and h == 7:
                    nc.sync.dma_start(zdbg_d[0:96, :], rz[:])
                    nc.sync.dma_start(zdbg_d[96:192, :], yg[:])
                    nc.sync.dma_start(zdbg_d[192:288, :], zg[:])

            if DEBUG_DUMP:
                for ci in range(CH):
                    nc.sync.dma_start(udbg_d[P * ci:P * (ci + 1), :],
                                      u_sb[ci][:])

            # ---- projection + bias ----
            for nt in range(4):
                ps, _ = psa_tile([P, C], name="pps")
                for half, w in ((0, 512), (512, 256)):
                    for ci in range(CH):
                        nc.tensor.matmul(
                            ps[:, half:half + w],
                            lhsT=u_sb[ci][:, P * nt:P * (nt + 1)],
                            rhs=wp[ci][:, half:half + w],
                            start=(ci == 0), stop=(ci == CH - 1))
                o_sb = outp.tile([P, C], f32, tag="osb")
                nc.vector.tensor_add(o_sb[:], ps[:], bias[:])
                nc.sync.dma_start(out_d[P * nt:P * (nt + 1), :], o_sb[:])

    nc.finalize()
    return nc


def _prep_inputs(x, W_qkv, W_proj, b_proj):
    bf16 = ml_dtypes.bfloat16
    # wq: per head [q4, q5, q5] (96 cols); wk: per head [k3, k5, k4]
    qcols, kcols = [], []
    for h in range(H):
        qb, kb = HD * h, C + HD * h
        qcols += list(range(qb + 32, qb + 64)) + 2 * list(range(qb + 64, qb + 96))
        kcols += (list(range(kb, kb + 32)) + list(range(kb + 64, kb + 96))
                  + list(range(kb + 32, kb + 64)))
    wq = np.ascontiguousarray(W_qkv[:, qcols]).astype(bf16)
    wk = np.ascontiguousarray(W_qkv[:, kcols]).astype(bf16)
    wv = np.ascontiguousarray(W_qkv[:, 2 * C:3 * C]).astype(bf16)
    # u channels on-chip are head-contiguous (96h + 32g + d); permute the
    # proj weight rows to match: wp[96h + 32g + d] = W_proj[256g + 32h + d].
    uperm = np.empty(C, np.int64)
    for hh in range(H):
        for g in range(3):
            uperm[96 * hh + 32 * g:96 * hh + 32 * g + 32] = np.arange(
                256 * g + 32 * hh, 256 * g + 32 * hh + 32)
    wp = np.ascontiguousarray(np.asarray(W_proj)[uperm, :]).astype(bf16)
    bias = np.broadcast_to(np.asarray(b_proj, np.float32), (P, C)).copy()

    in_maps = []
    for core in range(NCORES):
        b, half = core // 2, core % 2
        xb = np.asarray(x[b], np.float32)
        xp = np.concatenate([xb[NQ * half:NQ * (half + 1)],
                             xb[NQ * (1 - half):NQ * (2 - half)]], axis=0)
        xt = np.ascontiguousarray(xp.T).astype(bf16)
        in_maps.append({"xt": xt, "wq": wq, "wk": wk, "wv": wv, "wp": wp,
                        "bias": bias})
    return in_maps


def kernel(x, W_qkv, W_proj, b_proj, t_h=None, t_w=None, s_h=None, s_w=None,
           **_unused):
    from concourse.bass_utils import run_bass_kernel_spmd

    if "nc" not in _CACHE:
        _CACHE["nc"] = _build_graph()
    nc = _CACHE["nc"]

    in_maps = _prep_inputs(np.asarray(x), np.asarray(W_qkv),
                           np.asarray(W_proj), np.asarray(b_proj))
    res = run_bass_kernel_spmd(nc, in_maps, core_ids=list(range(NCORES)))
    _CACHE["last_results"] = res

    out = np.empty((B, N, C), np.float32)
    for core in range(NCORES):
        b, half = core // 2, core % 2
        out[b, NQ * half:NQ * (half + 1), :] = res.results[core]["out"]
    return out

